# revision 48
# baseline (speedup 1.0000x reference)
"""Trainium2 Bass kernel for nn_DecoderLSTM.

Key observation: the reference module never reads `features` -- the LSTM input
starts at zeros and is fed back from the predicted point, and h/c start at
zeros.  Every batch row therefore computes the *identical* trajectory
p[t] (t=0..83); the per-row output is just p[t] masked by t < seq_lengths[b].

The sequential 84-step scan is replaced by a parallel-in-time Gauss-Seidel
iteration: all 84 timesteps are updated simultaneously (matmuls with N=84
moving columns), and the linear cell-state recurrence
c_t = sig(f_t)*c_{t-1} + sig(i_t)*tanh(g_t) is solved exactly within each
sweep by the DVE's tensor_tensor_scan.  Convergence (verified against the
host oracle): rel err 2.3e-3 after 4 sweeps, 8.8e-4 after 5, 2.0e-4 after 6.
The sequential version pays the PE weight-load for every 128x128 tile at
every one of 84 steps (~40ns/tile, 216 tiles/step -> ~700us); here each
weight tile is loaded once per sweep and serves all 84 columns.

Hardware rules this code is shaped around:
  - walrus allows ONE sync-wait per instruction.  Tile chains PSUM readers
    with sync edges at tensor granularity, so every PSUM tensor has exactly
    one reader instruction per sweep (a DVE copy/bias-add); ACT never reads
    PSUM and every ACT-written tile is fresh per sweep (ACT-ACT WAW edges
    also emit waits).
  - only one PSUM accumulation group may be open per 2KB bank, so each gate
    region's matmuls are emitted as one tight group.
  - gates are grouped into PSUM tensors by gate type, ordered (g,f01 |
    f23,i | o) so the o-gate matmuls stream while the scan/tanh chain runs:
    the PE's post-cell stall is only gb_o -> sig_o -> h'.

Layouts (per core):
  - states H0s/H1s (ping-pong pair): [128, 4, 85] fp16; col tau holds
    h(tau-1), col 0 is the t=-1 zero state; chunk k on dim1: h[128k+p].
  - x feed Xs: [4, 85] fp16; rows 0:3 = x (col tau = p_{tau-1}), row 3 = 1
    (carries the cell-0 bias through the x matmul, K=4).
  - gate region r = q*4 + ml (gate q of (i,f,o,g), h-chunk ml), 84 columns
    each, distributed across three PSUM tensors per cell in SLOT order
    (12,13,14,15,4,5 | 6,7,0,1,2,3 | 8,9,10,11).
  - weights: lhsT tiles [K=128, M=128] fp16, m-major (free = m*512+k*128+j),
    DMA'd in 4 chunks ordered by first use.
"""

import numpy as np

B = 16384
H = 512
T = 84
IN = 3
N_CORES = 8
NB = B // N_CORES          # 2048 rows per core
M_TILES = 16               # 2048 gate dims / 128
BT = NB // 128             # 16 batch tiles per core
F_OUT = T * IN             # 252
J_SWEEPS = 4

# gate regions in emission (slot) order: tensor A = g gates + f chunks 0,1;
# tensor B = f chunks 2,3 + i gates; tensor O = o gates (streamed last)
SLOTS_A = (12, 13, 14, 15, 4, 5)
SLOTS_B = (6, 7, 0, 1, 2, 3)
SLOTS_O = (8, 9, 10, 11)
SLOTS = SLOTS_A + SLOTS_B + SLOTS_O
# weight DMA chunks (4 m-tiles each) in first-use order of SLOTS
WCHUNKS = ((12, 16), (4, 8), (0, 4), (8, 12))

_COMPILED = None           # nc cache
LAST_RESULTS = None        # BassKernelResults from the last run (for test.py)


def _gate_reorder(a, axis=0):
    """torch gate order (i,f,g,o) -> (i,f,o,g) along `axis` (size 4H)."""
    parts = np.split(a, 4, axis=axis)
    return np.concatenate([parts[0], parts[1], parts[3], parts[2]], axis=axis)


def _lhsT_tiles_mmajor(wT, kt):
    """wT: [K, 2048] -> [128, 16*kt*128] with free index (m, k, j)."""
    K = wT.shape[0]
    assert K == kt * 128
    a = wT.reshape(kt, 128, M_TILES, 128)       # [k, p, m, j]
    return np.ascontiguousarray(a.transpose(1, 2, 0, 3).reshape(128, kt * 2048))


def _build_program():
    import concourse.bass as bass
    import concourse.tile as tile
    import concourse.mybir as mybir
    from contextlib import ExitStack

    f16 = mybir.dt.float16
    f32 = mybir.dt.float32
    AF = mybir.ActivationFunctionType
    Alu = mybir.AluOpType

    class SplitDrainTileContext(tile.TileContext):
        """This container's walrus allows only one sync-wait per instruction;
        Tile's kernel-tail drain carries one wait per live semaphore.  Split
        it into a chain of single-wait drains (same semantics: by the last
        drain every semaphore has reached its target)."""

        def _drain_and_barrier(self, tick_clock, wait_clock):
            from concourse.vector_clock import ScopedClock
            drain_inst = self.nc.sync.drain()
            wait_clock.add_sem_waits(
                drain_inst.ins, ScopedClock({None: tick_clock.global_clock}))
            si = drain_inst.ins.sync_info
            waits = list(si.on_wait or []) if si is not None else []
            if len(waits) > 1:
                ups = list(si.on_update or [])
                drain_inst.ins.sync_info = mybir.SyncInfo(
                    on_wait=[waits[0]], on_update=ups)
                for w in waits[1:]:
                    d2 = self.nc.sync.drain()
                    d2.ins.sync_info = mybir.SyncInfo(on_wait=[w], on_update=[])
            self.nc.all_engine_barrier()
            popped = self.nc._tile_sem_poison_stack.pop()
            assert popped is self._sem_poison
            self.nc.clear_and_free_semaphores(list(self.sems.allocated().values()))
            self.nc.all_engine_barrier()

    nc = bass.Bass()

    w0T = nc.declare_dram_parameter("w0T", [128, 4 * 2048], f16, isOutput=False)
    w1iT = nc.declare_dram_parameter("w1iT", [128, 4 * 2048], f16, isOutput=False)
    w1hT = nc.declare_dram_parameter("w1hT", [128, 4 * 2048], f16, isOutput=False)
    wx4T = nc.declare_dram_parameter("wx4T", [4, 2048], f16, isOutput=False)
    wpT = nc.declare_dram_parameter("wpT", [128, 12], f16, isOutput=False)
    b1rd = nc.declare_dram_parameter("b1rep", [128, M_TILES * T], f16, isOutput=False)
    oh3d = nc.declare_dram_parameter("oh3", [3, 3 * 128], f16, isOutput=False)
    bpcd = nc.declare_dram_parameter("bpc84", [3, T], f32, isOutput=False)
    tvd = nc.declare_dram_parameter("tvals", [1, F_OUT], f16, isOutput=False)
    xsid = nc.declare_dram_parameter("xsinit", [4, T + 1], f16, isOutput=False)
    lensd = nc.declare_dram_parameter("lens", [NB], f16, isOutput=False)
    outd = nc.declare_dram_parameter("out", [NB, F_OUT], f32, isOutput=True)

    with ExitStack() as ctx:
        tc = ctx.enter_context(SplitDrainTileContext(nc))
        const = ctx.enter_context(tc.tile_pool(name="const", bufs=1))
        tmp = ctx.enter_context(tc.tile_pool(name="tmp", bufs=2))
        GA0 = ctx.enter_context(nc.psum_tensor([128, 1, 512], f32))
        GB0 = ctx.enter_context(nc.psum_tensor([128, 1, 512], f32))
        GO0 = ctx.enter_context(nc.psum_tensor([128, 1, 512], f32))
        GA1 = ctx.enter_context(nc.psum_tensor([128, 1, 512], f32))
        GB1 = ctx.enter_context(nc.psum_tensor([128, 1, 512], f32))
        GO1 = ctx.enter_context(nc.psum_tensor([128, 1, 512], f32))
        PB = ctx.enter_context(nc.psum_tensor([128, 1, 512], f32))
        BC = ctx.enter_context(nc.psum_tensor([128, 1, 512], f32))
        G0 = (GA0, GB0, GO0)
        G1 = (GA1, GB1, GO1)
        Pap = PB[0:3, 0, 0:T]              # head output

        _r2slot = {}
        for s, r in enumerate(SLOTS_A):
            _r2slot[r] = (0, s)
        for s, r in enumerate(SLOTS_B):
            _r2slot[r] = (1, s)
        for s, r in enumerate(SLOTS_O):
            _r2slot[r] = (2, s)

        def greg(G, r):
            ti, s = _r2slot[r]
            return G[ti][:, 0, s * T:(s + 1) * T]

        # ---- constants / weights into SBUF ----
        # weight DMAs are chunked and ordered by first use; each chunk's
        # semaphore is absorbed into the PE clock by a tiny ldweights right
        # before its first consuming matmul.
        # DMA ring-flow waits are cumulative per queue and walrus allows
        # only one sync wait per instruction, so the gpsimd queue is
        # reserved for the four output stores; loads alternate between the
        # SP and ACT queues so weight chunks land in parallel and sweeps
        # 0/1 are not DMA-starved (6.4MB of weights vs ~200GB/s per queue).
        queues = (nc.sync.dma_start, nc.scalar.dma_start)
        wx4s = const.tile([4, 2048], f16)
        queues[0](wx4s[:], wx4T[:, :])
        Xs = const.tile([4, T + 1], f16)
        queues[1](Xs[:], xsid[:, :])  # rows 0:3 zero, row 3 = 1 (bias)
        b1rs = const.tile([128, M_TILES * T], f16)
        queues[0](b1rs[:], b1rd[:, :])
        wpss = const.tile([128, 12], f16)
        queues[0](wpss[:], wpT[:, :])
        w1is = const.tile([128, 4 * 2048], f16)
        for qi, (lo, hi) in enumerate(WCHUNKS):
            queues[qi % 2](w1is[:, lo * 512:hi * 512], w1iT[:, lo * 512:hi * 512])
        w0s = const.tile([128, 4 * 2048], f16)
        for qi, (lo, hi) in enumerate(WCHUNKS):
            queues[(qi + 1) % 2](w0s[:, lo * 512:hi * 512], w0T[:, lo * 512:hi * 512])
        w1hs = const.tile([128, 4 * 2048], f16)
        for qi, (lo, hi) in enumerate(WCHUNKS):
            queues[qi % 2](w1hs[:, lo * 512:hi * 512], w1hT[:, lo * 512:hi * 512])
        oh3s = const.tile([3, 3 * 128], f16)
        queues[1](oh3s[:], oh3d[:, :])
        bpcs = const.tile([3, T], f32)
        queues[0](bpcs[:], bpcd[:, :])
        tvs = const.tile([1, F_OUT], f16)
        queues[0](tvs[:], tvd[:, :])
        lenss = const.tile([128, BT], f16)
        queues[1](lenss[:], lensd.rearrange("(m p) -> p m", p=128))

        ones1 = const.tile([1, 128], f16)
        nc.vector.memset(ones1[:], 1.0)

        # states: ping-pong buffers, col 0 = zero state
        H0s = [const.tile([128, 4, T + 1], f16, name=f"h0_{i}") for i in range(2)]
        H1s = [const.tile([128, 4, T + 1], f16, name=f"h1_{i}") for i in range(2)]
        for s in (*H0s, *H1s):
            nc.vector.memset(s[:, :, 0:1], 0.0)

        # DVE absorbers for DVE-consumed const DMAs
        absb = const.tile([1, 5], f32)
        nc.vector.tensor_copy(absb[:, 0:1], b1rs[0:1, 0:1])
        nc.vector.tensor_copy(absb[:, 1:2], bpcs[0:1, 0:1])
        nc.vector.tensor_copy(absb[:, 2:3], tvs[0:1, 0:1])
        nc.vector.tensor_copy(absb[:, 3:4], lenss[0:1, 0:1])
        nc.vector.tensor_copy(absb[:, 4:5], Xs[0:1, 0:1])
        nc.tensor.ldweights(wx4s[0:4, 0:1])

        # broadcast the t-values row across partitions (once)
        tvbc = const.tile([128, F_OUT], f16)
        nc.tensor.matmul(BC[:, 0, F_OUT:2 * F_OUT], lhsT=ones1[:], rhs=tvs[:],
                         start=True, stop=True)
        nc.vector.tensor_copy(tvbc[:], BC[:, 0, F_OUT:2 * F_OUT])
        ones252 = const.tile([128, F_OUT], f16)
        nc.vector.memset(ones252[:], 1.0)
        # per-row length masks (t < len), input-only: computed during the
        # sweeps while the DVE idles under the matmul blocks
        maskt = const.tile([128, BT, F_OUT], f16)

        def emit_masks(lo, hi):
            for n in range(lo, hi):
                nc.vector.scalar_tensor_tensor(
                    maskt[:, n, :], tvbc[:], lenss[:, n:n + 1],
                    ones252[:], Alu.is_lt, Alu.mult)

        def cell_elementwise_a(G, cellno, j, bias):
            """Part 1: after tensors A and B are complete -- tanh(g),
            sig(f,i), u, the c scan and tanh(c).  Runs while the o-gate
            matmuls stream through the PE."""
            GA, GB, _ = G
            gba = tmp.tile([128, 6 * T], f16, tag=f"gba{cellno}", bufs=2)
            if bias is not None:
                nc.vector.tensor_add(gba[:], GA[:, 0, 0:6 * T], bias[:, 0:6 * T])
            else:
                nc.vector.tensor_copy(gba[:], GA[:, 0, 0:6 * T])
            tg = tmp.tile([128, 4 * T], f16, tag=f"tg{cellno}_{j}", bufs=1)
            nc.scalar.activation(tg[:], gba[:, 0:4 * T], AF.Tanh)     # tanh(g)
            sgf01 = tmp.tile([128, 2 * T], f16, tag=f"sgf01_{cellno}_{j}", bufs=1)
            nc.scalar.activation(sgf01[:], gba[:, 4 * T:6 * T], AF.Sigmoid)
            gbb = tmp.tile([128, 6 * T], f16, tag=f"gbb{cellno}", bufs=2)
            if bias is not None:
                nc.vector.tensor_add(gbb[:], GB[:, 0, 0:6 * T], bias[:, 6 * T:12 * T])
            else:
                nc.vector.tensor_copy(gbb[:], GB[:, 0, 0:6 * T])
            # sigmoid of (f23, i01) first: chunk-0's u/scan only needs
            # those, so the c-chain starts before sig(i23) finishes
            sgb = tmp.tile([128, 4 * T], f16, tag=f"sgb{cellno}_{j}", bufs=1)
            nc.scalar.activation(sgb[:], gbb[:, 0:4 * T], AF.Sigmoid)
            sgc = tmp.tile([128, 2 * T], f16, tag=f"sgc{cellno}_{j}", bufs=1)
            nc.scalar.activation(sgc[:], gbb[:, 4 * T:6 * T], AF.Sigmoid)
            # per-chunk pipeline: u -> scan -> tanh(c) for chunk c runs
            # while chunk c+1 is still scanning, so tanh(c) of the last
            # chunk lands just after the last scan instead of after all four
            u = tmp.tile([128, 4 * T], f16, tag=f"u{cellno}", bufs=2)
            cf = tmp.tile([128, 4 * T], f16, tag=f"c{cellno}", bufs=2)
            tcns = []
            for c4, sf in ((0, sgf01[:, 0:T]), (1, sgf01[:, T:2 * T]),
                           (2, sgb[:, 0:T]), (3, sgb[:, T:2 * T])):
                cs = slice(c4 * T, (c4 + 1) * T)
                si_ap = (sgb[:, (2 + c4) * T:(3 + c4) * T] if c4 < 2
                         else sgc[:, (c4 - 2) * T:(c4 - 1) * T])
                nc.vector.tensor_mul(u[:, cs], si_ap, tg[:, cs])  # sig(i)*tanh(g)
                nc.vector.tensor_tensor_scan(
                    cf[:, cs], sf, u[:, cs],
                    0.0, Alu.mult, Alu.add)   # c_t = sig(f_t)*c_{t-1} + u_t
                tcn = tmp.tile([128, T], f16, tag=f"tc{cellno}_{j}_{c4}", bufs=1,
                               name=f"tcn{cellno}_{j}_{c4}")
                nc.scalar.activation(tcn[:], cf[:, cs], AF.Tanh)
                tcns.append(tcn)
            return tcns

        def cell_elementwise_o(G, tcn, h_out, cellno, j, bias):
            """Part 2: after tensor O -- sig(o), h' = sig(o)*tanh(c)."""
            _, _, GO = G
            gbo = tmp.tile([128, 4 * T], f16, tag=f"gbo{cellno}", bufs=2)
            if bias is not None:
                nc.vector.tensor_add(gbo[:], GO[:, 0, 0:4 * T], bias[:, 12 * T:])
            else:
                nc.vector.tensor_copy(gbo[:], GO[:, 0, 0:4 * T])
            sgo = tmp.tile([128, 4 * T], f16, tag=f"sgo{cellno}_{j}", bufs=1)
            nc.scalar.activation(sgo[:], gbo[:], AF.Sigmoid)
            # per-chunk h' so the first W_ih matmul (which consumes chunk 0)
            # can start before chunks 1-3 are written
            for c4 in range(4):
                nc.vector.tensor_mul(h_out[:, c4, 1:T + 1],
                                     sgo[:, c4 * T:(c4 + 1) * T], tcn[c4][:])

        def emit_head(h1buf):
            """P(:, t) = W_pc @ h1(t), then Xs[:, 1:] = P + b_pc (fp16)."""
            for k in range(4):
                nc.tensor.matmul(Pap, lhsT=wpss[:, 3 * k:3 * k + 3],
                                 rhs=h1buf[:, k, 1:T + 1],
                                 start=(k == 0), stop=(k == 3))
            nc.vector.tensor_add(Xs[0:3, 1:T + 1], Pap, bpcs[:])

        def absorb(ws, m):
            # absorb the DMA chunk whose first m-tile is m into the PE clock
            nc.tensor.ldweights(ws[:, m * 512:m * 512 + 1])

        for j in range(J_SWEEPS):
            r, w = j % 2, (j + 1) % 2
            if j > 0:
                if j == 1:
                    absorb(wpss, 0)
                emit_head(H1s[r])           # head of sweep j-1 -> x for sweep j
            # G0 = W_hh0 @ h0(prev, shifted) + W_ih0 @ x + b0 (K=4 x+bias
            # pass); one tight accumulation group per gate region.
            for si, m in enumerate(SLOTS):
                if j == 1 and si in (0, 4, 8, 12):
                    absorb(w0s, (m // 4) * 4)
                for k in range(4):
                    if j > 0:
                        nc.tensor.matmul(
                            greg(G0, m),
                            lhsT=w0s[:, m * 512 + k * 128:m * 512 + (k + 1) * 128],
                            rhs=H0s[r][:, k, 0:T],
                            start=(k == 0), stop=False)
                nc.tensor.matmul(
                    greg(G0, m),
                    lhsT=wx4s[0:4, m * 128:(m + 1) * 128],
                    rhs=Xs[0:4, 0:T],
                    start=(j == 0), stop=True)
                if si == len(SLOTS_A) + len(SLOTS_B) - 1:
                    tcn0 = cell_elementwise_a(G0, 0, j, None)
            cell_elementwise_o(G0, tcn0, H0s[w], 0, j, None)
            # G1 = W_ih1 @ h0(this sweep) + W_hh1 @ h1(prev, shifted)
            for si, m in enumerate(SLOTS):
                if j == 0 and si in (0, 4, 8, 12):
                    absorb(w1is, (m // 4) * 4)
                if j == 1 and si in (0, 4, 8, 12):
                    absorb(w1hs, (m // 4) * 4)
                for k in range(4):
                    nc.tensor.matmul(
                        greg(G1, m),
                        lhsT=w1is[:, m * 512 + k * 128:m * 512 + (k + 1) * 128],
                        rhs=H0s[w][:, k, 1:T + 1],
                        start=(k == 0), stop=(j == 0 and k == 3))
                if j > 0:
                    for k in range(4):
                        nc.tensor.matmul(
                            greg(G1, m),
                            lhsT=w1hs[:, m * 512 + k * 128:m * 512 + (k + 1) * 128],
                            rhs=H1s[r][:, k, 0:T],
                            start=False, stop=(k == 3))
                if si == len(SLOTS_A) + len(SLOTS_B) - 1:
                    tcn1 = cell_elementwise_a(G1, 1, j, b1rs)
            cell_elementwise_o(G1, tcn1, H1s[w], 1, j, b1rs)
            if j < 2:
                emit_masks(j * 8, (j + 1) * 8)

        # final head -> Xs[0:3, 1:85] = final trajectory points (with bias)
        emit_head(H1s[J_SWEEPS % 2])

        # ---- broadcast + mask + store ----
        nc.tensor.ldweights(oh3s[0:3, 0:1])   # absorb oh3 DMA sem into PE
        for i in range(3):
            nc.tensor.matmul(BC[:, 0, i * T:(i + 1) * T],
                             lhsT=oh3s[0:3, i * 128:(i + 1) * 128],
                             rhs=Xs[0:3, 1:T + 1], start=True, stop=True)
        # trajectory replicated x4 (interleaved [t, i] -> col 3t+i), so the
        # masking is 4 full-width fp16 multiplies instead of 16 small ops
        pq = const.tile([128, T, 3], f16)
        for i in range(3):
            nc.vector.tensor_copy(pq[:, :, i], BC[:, 0, i * T:(i + 1) * T])
        # stride-0 broadcast view: each masked multiply reads the same
        # trajectory for its 4 batch tiles, no physical replication
        pqr = pq[:, :, :].unsqueeze(1).broadcast_to([128, 4, T, 3])
        ot = const.tile([128, BT * F_OUT], f16)
        out_r = outd.rearrange("(n p) f -> p n f", p=128)
        for n4 in range(4):
            nc.vector.tensor_mul(
                ot[:, n4 * 4 * F_OUT:(n4 + 1) * 4 * F_OUT],
                maskt[:, n4 * 4:(n4 + 1) * 4, :], pqr)
            # the gpsimd (software DGE) store casts fp16 -> fp32 in flight;
            # the fp16 path is bit-exact because the trajectory already
            # passed through the fp16 Xs tile
            nc.gpsimd.dma_start(
                out_r[:, n4 * 4:(n4 + 1) * 4, :],
                ot[:, n4 * 4 * F_OUT:(n4 + 1) * 4 * F_OUT])

    return nc


def _prep_inputs(inputs):
    f = lambda k: np.asarray(inputs[k], np.float32)
    Wih0 = _gate_reorder(f("W_ih0"))
    Whh0 = _gate_reorder(f("W_hh0"))
    Wih1 = _gate_reorder(f("W_ih1"))
    Whh1 = _gate_reorder(f("W_hh1"))
    b0 = _gate_reorder(f("b_ih0") + f("b_hh0"))
    b1 = _gate_reorder(f("b_ih1") + f("b_hh1"))
    Wpc = f("W_pc")
    bpc = f("b_pc")

    oh3 = np.zeros((3, 3 * 128), np.float16)
    for i in range(3):
        oh3[i, i * 128:(i + 1) * 128] = 1.0

    # b1 replicated across T in SLOT order: col s*T+t = b1[SLOTS[s]*128+p]
    b1m = b1.reshape(M_TILES, 128)[list(SLOTS)]           # [16 slots, 128]
    b1rep = np.repeat(b1m.T[:, :, None], T, axis=2).reshape(128, M_TILES * T)

    common = {
        "w0T": _lhsT_tiles_mmajor(Whh0.T.copy(), 4).astype(np.float16),
        "w1iT": _lhsT_tiles_mmajor(Wih1.T.copy(), 4).astype(np.float16),
        "w1hT": _lhsT_tiles_mmajor(Whh1.T.copy(), 4).astype(np.float16),
        "wx4T": np.ascontiguousarray(
            np.concatenate([Wih0.T, b0[None, :]], 0)).astype(np.float16),
        "wpT": np.ascontiguousarray(
            Wpc.T.reshape(4, 128, 3).transpose(1, 0, 2).reshape(128, 12)
        ).astype(np.float16),
        "b1rep": np.ascontiguousarray(b1rep).astype(np.float16),
        "oh3": oh3,
        "bpc84": np.ascontiguousarray(np.repeat(bpc[:, None], T, axis=1)),
        "tvals": np.repeat(np.arange(T, dtype=np.float16), IN).reshape(1, F_OUT),
        "xsinit": np.concatenate(
            [np.zeros((3, T + 1), np.float16), np.ones((1, T + 1), np.float16)], 0),
    }
    lens = np.asarray(inputs["seq_lengths"]).astype(np.float16)
    in_maps = []
    for c in range(N_CORES):
        m = dict(common)
        m["lens"] = np.ascontiguousarray(lens[c * NB:(c + 1) * NB])
        in_maps.append(m)
    return in_maps


def kernel(**inputs):
    global _COMPILED, LAST_RESULTS
    from concourse.bass_utils import run_bass_kernel_spmd

    if _COMPILED is None:
        _COMPILED = _build_program()
    nc = _COMPILED

    in_maps = _prep_inputs(inputs)
    res = run_bass_kernel_spmd(nc, in_maps, list(range(N_CORES)))
    LAST_RESULTS = res
    out = np.concatenate([res.results[c]["out"] for c in range(N_CORES)], axis=0)
    return np.ascontiguousarray(out.reshape(B, T, IN))


# revision 50
# speedup vs baseline: 1.0013x; 1.0013x over previous
"""Trainium2 Bass kernel for nn_DecoderLSTM.

Key observation: the reference module never reads `features` -- the LSTM input
starts at zeros and is fed back from the predicted point, and h/c start at
zeros.  Every batch row therefore computes the *identical* trajectory
p[t] (t=0..83); the per-row output is just p[t] masked by t < seq_lengths[b].

The sequential 84-step scan is replaced by a parallel-in-time Gauss-Seidel
iteration: all 84 timesteps are updated simultaneously (matmuls with N=84
moving columns), and the linear cell-state recurrence
c_t = sig(f_t)*c_{t-1} + sig(i_t)*tanh(g_t) is solved exactly within each
sweep by the DVE's tensor_tensor_scan.  Convergence (verified against the
host oracle): rel err 2.3e-3 after 4 sweeps, 8.8e-4 after 5, 2.0e-4 after 6.
The sequential version pays the PE weight-load for every 128x128 tile at
every one of 84 steps (~40ns/tile, 216 tiles/step -> ~700us); here each
weight tile is loaded once per sweep and serves all 84 columns.

Hardware rules this code is shaped around:
  - walrus allows ONE sync-wait per instruction.  Tile chains PSUM readers
    with sync edges at tensor granularity, so every PSUM tensor has exactly
    one reader instruction per sweep (a DVE copy/bias-add); ACT never reads
    PSUM and every ACT-written tile is fresh per sweep (ACT-ACT WAW edges
    also emit waits).
  - only one PSUM accumulation group may be open per 2KB bank, so each gate
    region's matmuls are emitted as one tight group.
  - gates are grouped into PSUM tensors by gate type, ordered (g,f01 |
    f23,i | o) so the o-gate matmuls stream while the scan/tanh chain runs:
    the PE's post-cell stall is only gb_o -> sig_o -> h'.

Layouts (per core):
  - states H0s/H1s (ping-pong pair): [128, 4, 85] fp16; col tau holds
    h(tau-1), col 0 is the t=-1 zero state; chunk k on dim1: h[128k+p].
  - x feed Xs: [4, 85] fp16; rows 0:3 = x (col tau = p_{tau-1}), row 3 = 1
    (carries the cell-0 bias through the x matmul, K=4).
  - gate region r = q*4 + ml (gate q of (i,f,o,g), h-chunk ml), 84 columns
    each, distributed across three PSUM tensors per cell in SLOT order
    (12,13,14,15,4,5 | 6,7,0,1,2,3 | 8,9,10,11).
  - weights: lhsT tiles [K=128, M=128] fp16, m-major (free = m*512+k*128+j),
    DMA'd in 4 chunks ordered by first use.
"""

import numpy as np

B = 16384
H = 512
T = 84
IN = 3
N_CORES = 8
NB = B // N_CORES          # 2048 rows per core
M_TILES = 16               # 2048 gate dims / 128
BT = NB // 128             # 16 batch tiles per core
F_OUT = T * IN             # 252
J_SWEEPS = 4

# gate regions in emission (slot) order: tensor A = g gates + f chunks 0,1;
# tensor B = f chunks 2,3 + i gates; tensor O = o gates (streamed last)
SLOTS_A = (12, 13, 14, 15, 4, 5)
SLOTS_B = (6, 7, 0, 1, 2, 3)
SLOTS_O = (8, 9, 10, 11)
SLOTS = SLOTS_A + SLOTS_B + SLOTS_O
# weight DMA chunks (4 m-tiles each) in first-use order of SLOTS
WCHUNKS = ((12, 16), (4, 8), (0, 4), (8, 12))

_COMPILED = None           # nc cache
LAST_RESULTS = None        # BassKernelResults from the last run (for test.py)


def _gate_reorder(a, axis=0):
    """torch gate order (i,f,g,o) -> (i,f,o,g) along `axis` (size 4H)."""
    parts = np.split(a, 4, axis=axis)
    return np.concatenate([parts[0], parts[1], parts[3], parts[2]], axis=axis)


def _lhsT_tiles_mmajor(wT, kt):
    """wT: [K, 2048] -> [128, 16*kt*128] with free index (m, k, j)."""
    K = wT.shape[0]
    assert K == kt * 128
    a = wT.reshape(kt, 128, M_TILES, 128)       # [k, p, m, j]
    return np.ascontiguousarray(a.transpose(1, 2, 0, 3).reshape(128, kt * 2048))


def _build_program():
    import concourse.bass as bass
    import concourse.tile as tile
    import concourse.mybir as mybir
    from contextlib import ExitStack

    f16 = mybir.dt.float16
    f32 = mybir.dt.float32
    AF = mybir.ActivationFunctionType
    Alu = mybir.AluOpType

    class SplitDrainTileContext(tile.TileContext):
        """This container's walrus allows only one sync-wait per instruction;
        Tile's kernel-tail drain carries one wait per live semaphore.  Split
        it into a chain of single-wait drains (same semantics: by the last
        drain every semaphore has reached its target)."""

        def _drain_and_barrier(self, tick_clock, wait_clock):
            from concourse.vector_clock import ScopedClock
            drain_inst = self.nc.sync.drain()
            wait_clock.add_sem_waits(
                drain_inst.ins, ScopedClock({None: tick_clock.global_clock}))
            si = drain_inst.ins.sync_info
            waits = list(si.on_wait or []) if si is not None else []
            if len(waits) > 1:
                ups = list(si.on_update or [])
                drain_inst.ins.sync_info = mybir.SyncInfo(
                    on_wait=[waits[0]], on_update=ups)
                for w in waits[1:]:
                    d2 = self.nc.sync.drain()
                    d2.ins.sync_info = mybir.SyncInfo(on_wait=[w], on_update=[])
            self.nc.all_engine_barrier()
            popped = self.nc._tile_sem_poison_stack.pop()
            assert popped is self._sem_poison
            self.nc.clear_and_free_semaphores(list(self.sems.allocated().values()))
            self.nc.all_engine_barrier()

    nc = bass.Bass()

    w0T = nc.declare_dram_parameter("w0T", [128, 4 * 2048], f16, isOutput=False)
    w1iT = nc.declare_dram_parameter("w1iT", [128, 4 * 2048], f16, isOutput=False)
    w1hT = nc.declare_dram_parameter("w1hT", [128, 4 * 2048], f16, isOutput=False)
    wx4T = nc.declare_dram_parameter("wx4T", [4, 2048], f16, isOutput=False)
    wpT = nc.declare_dram_parameter("wpT", [128, 12], f16, isOutput=False)
    b1rd = nc.declare_dram_parameter("b1rep", [128, M_TILES * T], f16, isOutput=False)
    oh3d = nc.declare_dram_parameter("oh3", [3, 3 * 128], f16, isOutput=False)
    bpcd = nc.declare_dram_parameter("bpc84", [3, T], f32, isOutput=False)
    tvd = nc.declare_dram_parameter("tvals", [1, F_OUT], f16, isOutput=False)
    xsid = nc.declare_dram_parameter("xsinit", [4, T + 1], f16, isOutput=False)
    lensd = nc.declare_dram_parameter("lens", [NB], f16, isOutput=False)
    outd = nc.declare_dram_parameter("out", [NB, F_OUT], f32, isOutput=True)

    with ExitStack() as ctx:
        tc = ctx.enter_context(SplitDrainTileContext(nc))
        const = ctx.enter_context(tc.tile_pool(name="const", bufs=1))
        tmp = ctx.enter_context(tc.tile_pool(name="tmp", bufs=2))
        GA0 = ctx.enter_context(nc.psum_tensor([128, 1, 512], f32))
        GB0 = ctx.enter_context(nc.psum_tensor([128, 1, 512], f32))
        GO0 = ctx.enter_context(nc.psum_tensor([128, 1, 512], f32))
        GA1 = ctx.enter_context(nc.psum_tensor([128, 1, 512], f32))
        GB1 = ctx.enter_context(nc.psum_tensor([128, 1, 512], f32))
        GO1 = ctx.enter_context(nc.psum_tensor([128, 1, 512], f32))
        PB = ctx.enter_context(nc.psum_tensor([128, 1, 512], f32))
        BC = ctx.enter_context(nc.psum_tensor([128, 1, 512], f32))
        G0 = (GA0, GB0, GO0)
        G1 = (GA1, GB1, GO1)
        Pap = PB[0:3, 0, 0:T]              # head output

        _r2slot = {}
        for s, r in enumerate(SLOTS_A):
            _r2slot[r] = (0, s)
        for s, r in enumerate(SLOTS_B):
            _r2slot[r] = (1, s)
        for s, r in enumerate(SLOTS_O):
            _r2slot[r] = (2, s)

        def greg(G, r):
            ti, s = _r2slot[r]
            return G[ti][:, 0, s * T:(s + 1) * T]

        # ---- constants / weights into SBUF ----
        # weight DMAs are chunked and ordered by first use; each chunk's
        # semaphore is absorbed into the PE clock by a tiny ldweights right
        # before its first consuming matmul.
        # DMA ring-flow waits are cumulative per queue and walrus allows
        # only one sync wait per instruction, so the gpsimd queue is
        # reserved for the four output stores; loads alternate between the
        # SP and ACT queues so weight chunks land in parallel and sweeps
        # 0/1 are not DMA-starved (6.4MB of weights vs ~200GB/s per queue).
        queues = (nc.sync.dma_start, nc.scalar.dma_start)
        wx4s = const.tile([4, 2048], f16)
        queues[0](wx4s[:], wx4T[:, :])
        Xs = const.tile([4, T + 1], f16)
        queues[1](Xs[:], xsid[:, :])  # rows 0:3 zero, row 3 = 1 (bias)
        b1rs = const.tile([128, M_TILES * T], f16)
        queues[0](b1rs[:], b1rd[:, :])
        wpss = const.tile([128, 12], f16)
        queues[0](wpss[:], wpT[:, :])
        w1is = const.tile([128, 4 * 2048], f16)
        for qi, (lo, hi) in enumerate(WCHUNKS):
            queues[qi % 2](w1is[:, lo * 512:hi * 512], w1iT[:, lo * 512:hi * 512])
        w0s = const.tile([128, 4 * 2048], f16)
        for qi, (lo, hi) in enumerate(WCHUNKS):
            queues[(qi + 1) % 2](w0s[:, lo * 512:hi * 512], w0T[:, lo * 512:hi * 512])
        w1hs = const.tile([128, 4 * 2048], f16)
        for qi, (lo, hi) in enumerate(WCHUNKS):
            queues[qi % 2](w1hs[:, lo * 512:hi * 512], w1hT[:, lo * 512:hi * 512])
        oh3s = const.tile([3, 3 * 128], f16)
        queues[1](oh3s[:], oh3d[:, :])
        bpcs = const.tile([3, T], f32)
        queues[0](bpcs[:], bpcd[:, :])
        tvs = const.tile([1, F_OUT], f16)
        queues[0](tvs[:], tvd[:, :])
        lenss = const.tile([128, BT], f16)
        queues[1](lenss[:], lensd.rearrange("(m p) -> p m", p=128))

        ones1 = const.tile([1, 128], f16)
        nc.vector.memset(ones1[:], 1.0)

        # states: ping-pong buffers, col 0 = zero state
        H0s = [const.tile([128, 4, T + 1], f16, name=f"h0_{i}") for i in range(2)]
        H1s = [const.tile([128, 4, T + 1], f16, name=f"h1_{i}") for i in range(2)]
        for s in (*H0s, *H1s):
            nc.vector.memset(s[:, :, 0:1], 0.0)

        # DVE absorbers for DVE-consumed const DMAs
        absb = const.tile([1, 5], f32)
        nc.vector.tensor_copy(absb[:, 0:1], b1rs[0:1, 0:1])
        nc.vector.tensor_copy(absb[:, 1:2], bpcs[0:1, 0:1])
        nc.vector.tensor_copy(absb[:, 2:3], tvs[0:1, 0:1])
        nc.vector.tensor_copy(absb[:, 3:4], lenss[0:1, 0:1])
        nc.vector.tensor_copy(absb[:, 4:5], Xs[0:1, 0:1])
        nc.tensor.ldweights(wx4s[0:4, 0:1])

        # broadcast the t-values row across partitions (once)
        tvbc = const.tile([128, F_OUT], f16)
        nc.tensor.matmul(BC[:, 0, F_OUT:2 * F_OUT], lhsT=ones1[:], rhs=tvs[:],
                         start=True, stop=True)
        nc.vector.tensor_copy(tvbc[:], BC[:, 0, F_OUT:2 * F_OUT])
        ones252 = const.tile([128, F_OUT], f16)
        nc.vector.memset(ones252[:], 1.0)
        # per-row length masks (t < len), input-only: computed during the
        # sweeps while the DVE idles under the matmul blocks
        maskt = const.tile([128, BT, F_OUT], f16)

        def emit_masks(lo, hi):
            for n in range(lo, hi):
                nc.vector.scalar_tensor_tensor(
                    maskt[:, n, :], tvbc[:], lenss[:, n:n + 1],
                    ones252[:], Alu.is_lt, Alu.mult)

        def cell_elementwise_a(G, cellno, j, bias):
            """Part 1: after tensors A and B are complete -- tanh(g),
            sig(f,i), u, the c scan and tanh(c).  Runs while the o-gate
            matmuls stream through the PE."""
            GA, GB, _ = G
            gba = tmp.tile([128, 6 * T], f16, tag=f"gba{cellno}", bufs=2)
            if bias is not None:
                nc.vector.tensor_add(gba[:], GA[:, 0, 0:6 * T], bias[:, 0:6 * T])
            else:
                nc.vector.tensor_copy(gba[:], GA[:, 0, 0:6 * T])
            tg = tmp.tile([128, 4 * T], f16, tag=f"tg{cellno}_{j}", bufs=1)
            nc.scalar.activation(tg[:], gba[:, 0:4 * T], AF.Tanh)     # tanh(g)
            sgf01 = tmp.tile([128, 2 * T], f16, tag=f"sgf01_{cellno}_{j}", bufs=1)
            nc.scalar.activation(sgf01[:], gba[:, 4 * T:6 * T], AF.Sigmoid)
            gbb = tmp.tile([128, 6 * T], f16, tag=f"gbb{cellno}", bufs=2)
            if bias is not None:
                nc.vector.tensor_add(gbb[:], GB[:, 0, 0:6 * T], bias[:, 6 * T:12 * T])
            else:
                nc.vector.tensor_copy(gbb[:], GB[:, 0, 0:6 * T])
            # sigmoid of (f23, i01) first: chunk-0's u/scan only needs
            # those, so the c-chain starts before sig(i23) finishes
            sgb = tmp.tile([128, 4 * T], f16, tag=f"sgb{cellno}_{j}", bufs=1)
            nc.scalar.activation(sgb[:], gbb[:, 0:4 * T], AF.Sigmoid)
            sgc = tmp.tile([128, 2 * T], f16, tag=f"sgc{cellno}_{j}", bufs=1)
            nc.scalar.activation(sgc[:], gbb[:, 4 * T:6 * T], AF.Sigmoid)
            # per-chunk pipeline: u -> scan -> tanh(c) for chunk c runs
            # while chunk c+1 is still scanning, so tanh(c) of the last
            # chunk lands just after the last scan instead of after all four
            u = tmp.tile([128, 4 * T], f16, tag=f"u{cellno}", bufs=2)
            cf = tmp.tile([128, 4 * T], f16, tag=f"c{cellno}", bufs=2)
            tcns = []
            for c4, sf in ((0, sgf01[:, 0:T]), (1, sgf01[:, T:2 * T]),
                           (2, sgb[:, 0:T]), (3, sgb[:, T:2 * T])):
                cs = slice(c4 * T, (c4 + 1) * T)
                si_ap = (sgb[:, (2 + c4) * T:(3 + c4) * T] if c4 < 2
                         else sgc[:, (c4 - 2) * T:(c4 - 1) * T])
                nc.vector.tensor_mul(u[:, cs], si_ap, tg[:, cs])  # sig(i)*tanh(g)
                nc.vector.tensor_tensor_scan(
                    cf[:, cs], sf, u[:, cs],
                    0.0, Alu.mult, Alu.add)   # c_t = sig(f_t)*c_{t-1} + u_t
                tcn = tmp.tile([128, T], f16, tag=f"tc{cellno}_{j}_{c4}", bufs=1,
                               name=f"tcn{cellno}_{j}_{c4}")
                nc.scalar.activation(tcn[:], cf[:, cs], AF.Tanh)
                tcns.append(tcn)
            return tcns

        def cell_elementwise_o(G, tcn, h_out, cellno, j, bias):
            """Part 2: after tensor O -- sig(o), h' = sig(o)*tanh(c)."""
            _, _, GO = G
            gbo = tmp.tile([128, 4 * T], f16, tag=f"gbo{cellno}", bufs=2)
            if bias is not None:
                nc.vector.tensor_add(gbo[:], GO[:, 0, 0:4 * T], bias[:, 12 * T:])
            else:
                nc.vector.tensor_copy(gbo[:], GO[:, 0, 0:4 * T])
            sgo = tmp.tile([128, 4 * T], f16, tag=f"sgo{cellno}_{j}", bufs=1)
            nc.scalar.activation(sgo[:], gbo[:], AF.Sigmoid)
            # per-chunk h' so the first W_ih matmul (which consumes chunk 0)
            # can start before chunks 1-3 are written
            for c4 in range(4):
                nc.vector.tensor_mul(h_out[:, c4, 1:T + 1],
                                     sgo[:, c4 * T:(c4 + 1) * T], tcn[c4][:])

        def emit_head(h1buf):
            """P(:, t) = W_pc @ h1(t), then Xs[:, 1:] = P + b_pc (fp16)."""
            for k in range(4):
                nc.tensor.matmul(Pap, lhsT=wpss[:, 3 * k:3 * k + 3],
                                 rhs=h1buf[:, k, 1:T + 1],
                                 start=(k == 0), stop=(k == 3))
            nc.vector.tensor_add(Xs[0:3, 1:T + 1], Pap, bpcs[:])

        def absorb(ws, m):
            # absorb the DMA chunk whose first m-tile is m into the PE clock
            nc.tensor.ldweights(ws[:, m * 512:m * 512 + 1])

        for j in range(J_SWEEPS):
            r, w = j % 2, (j + 1) % 2
            if j > 0:
                if j == 1:
                    absorb(wpss, 0)
                emit_head(H1s[r])           # head of sweep j-1 -> x for sweep j
            # G0 = W_hh0 @ h0(prev, shifted) + W_ih0 @ x + b0 (K=4 x+bias
            # pass); one tight accumulation group per gate region.
            for si, m in enumerate(SLOTS):
                if j == 1 and si in (0, 4, 8, 12):
                    absorb(w0s, (m // 4) * 4)
                for k in range(4):
                    if j > 0:
                        nc.tensor.matmul(
                            greg(G0, m),
                            lhsT=w0s[:, m * 512 + k * 128:m * 512 + (k + 1) * 128],
                            rhs=H0s[r][:, k, 0:T],
                            start=(k == 0), stop=False)
                nc.tensor.matmul(
                    greg(G0, m),
                    lhsT=wx4s[0:4, m * 128:(m + 1) * 128],
                    rhs=Xs[0:4, 0:T],
                    start=(j == 0), stop=True)
                if si == len(SLOTS_A) + len(SLOTS_B) - 1:
                    tcn0 = cell_elementwise_a(G0, 0, j, None)
            cell_elementwise_o(G0, tcn0, H0s[w], 0, j, None)
            # G1 = W_ih1 @ h0(this sweep) + W_hh1 @ h1(prev, shifted)
            for si, m in enumerate(SLOTS):
                if j == 0 and si in (0, 4, 8, 12):
                    absorb(w1is, (m // 4) * 4)
                if j == 1 and si in (0, 4, 8, 12):
                    absorb(w1hs, (m // 4) * 4)
                for k in range(4):
                    nc.tensor.matmul(
                        greg(G1, m),
                        lhsT=w1is[:, m * 512 + k * 128:m * 512 + (k + 1) * 128],
                        rhs=H0s[w][:, k, 1:T + 1],
                        start=(k == 0), stop=(j == 0 and k == 3))
                if j > 0:
                    for k in range(4):
                        nc.tensor.matmul(
                            greg(G1, m),
                            lhsT=w1hs[:, m * 512 + k * 128:m * 512 + (k + 1) * 128],
                            rhs=H1s[r][:, k, 0:T],
                            start=False, stop=(k == 3))
                if si == len(SLOTS_A) + len(SLOTS_B) - 1:
                    tcn1 = cell_elementwise_a(G1, 1, j, b1rs)
            cell_elementwise_o(G1, tcn1, H1s[w], 1, j, b1rs)
            if j < 3:
                # batches sized to hide under the next sweep's A-block on the
                # in-order DVE without delaying its Xs-add (and none in the
                # last sweep, whose tail feeds the epilogue)
                lo, hi = ((0, 6), (6, 11), (11, 16))[j]
                emit_masks(lo, hi)

        # final head -> Xs[0:3, 1:85] = final trajectory points (with bias)
        emit_head(H1s[J_SWEEPS % 2])

        # ---- broadcast + mask + store ----
        nc.tensor.ldweights(oh3s[0:3, 0:1])   # absorb oh3 DMA sem into PE
        for i in range(3):
            nc.tensor.matmul(BC[:, 0, i * T:(i + 1) * T],
                             lhsT=oh3s[0:3, i * 128:(i + 1) * 128],
                             rhs=Xs[0:3, 1:T + 1], start=True, stop=True)
        # trajectory replicated x4 (interleaved [t, i] -> col 3t+i), so the
        # masking is 4 full-width fp16 multiplies instead of 16 small ops
        pqr = const.tile([128, 4, T, 3], f16)
        for i in range(3):
            nc.vector.tensor_copy(pqr[:, 0, :, i], BC[:, 0, i * T:(i + 1) * T])
        for c in range(1, 4):
            nc.vector.tensor_copy(pqr[:, c, :, :], pqr[:, 0, :, :])
        ot = const.tile([128, BT * F_OUT], f16)
        out_r = outd.rearrange("(n p) f -> p n f", p=128)
        for n4 in range(4):
            nc.vector.tensor_mul(
                ot[:, n4 * 4 * F_OUT:(n4 + 1) * 4 * F_OUT],
                maskt[:, n4 * 4:(n4 + 1) * 4, :], pqr[:, :, :, :])
            # the gpsimd (software DGE) store casts fp16 -> fp32 in flight;
            # the fp16 path is bit-exact because the trajectory already
            # passed through the fp16 Xs tile
            nc.gpsimd.dma_start(
                out_r[:, n4 * 4:(n4 + 1) * 4, :],
                ot[:, n4 * 4 * F_OUT:(n4 + 1) * 4 * F_OUT])

    return nc


def _prep_inputs(inputs):
    f = lambda k: np.asarray(inputs[k], np.float32)
    Wih0 = _gate_reorder(f("W_ih0"))
    Whh0 = _gate_reorder(f("W_hh0"))
    Wih1 = _gate_reorder(f("W_ih1"))
    Whh1 = _gate_reorder(f("W_hh1"))
    b0 = _gate_reorder(f("b_ih0") + f("b_hh0"))
    b1 = _gate_reorder(f("b_ih1") + f("b_hh1"))
    Wpc = f("W_pc")
    bpc = f("b_pc")

    oh3 = np.zeros((3, 3 * 128), np.float16)
    for i in range(3):
        oh3[i, i * 128:(i + 1) * 128] = 1.0

    # b1 replicated across T in SLOT order: col s*T+t = b1[SLOTS[s]*128+p]
    b1m = b1.reshape(M_TILES, 128)[list(SLOTS)]           # [16 slots, 128]
    b1rep = np.repeat(b1m.T[:, :, None], T, axis=2).reshape(128, M_TILES * T)

    common = {
        "w0T": _lhsT_tiles_mmajor(Whh0.T.copy(), 4).astype(np.float16),
        "w1iT": _lhsT_tiles_mmajor(Wih1.T.copy(), 4).astype(np.float16),
        "w1hT": _lhsT_tiles_mmajor(Whh1.T.copy(), 4).astype(np.float16),
        "wx4T": np.ascontiguousarray(
            np.concatenate([Wih0.T, b0[None, :]], 0)).astype(np.float16),
        "wpT": np.ascontiguousarray(
            Wpc.T.reshape(4, 128, 3).transpose(1, 0, 2).reshape(128, 12)
        ).astype(np.float16),
        "b1rep": np.ascontiguousarray(b1rep).astype(np.float16),
        "oh3": oh3,
        "bpc84": np.ascontiguousarray(np.repeat(bpc[:, None], T, axis=1)),
        "tvals": np.repeat(np.arange(T, dtype=np.float16), IN).reshape(1, F_OUT),
        "xsinit": np.concatenate(
            [np.zeros((3, T + 1), np.float16), np.ones((1, T + 1), np.float16)], 0),
    }
    lens = np.asarray(inputs["seq_lengths"]).astype(np.float16)
    in_maps = []
    for c in range(N_CORES):
        m = dict(common)
        m["lens"] = np.ascontiguousarray(lens[c * NB:(c + 1) * NB])
        in_maps.append(m)
    return in_maps


def kernel(**inputs):
    global _COMPILED, LAST_RESULTS
    from concourse.bass_utils import run_bass_kernel_spmd

    if _COMPILED is None:
        _COMPILED = _build_program()
    nc = _COMPILED

    in_maps = _prep_inputs(inputs)
    res = run_bass_kernel_spmd(nc, in_maps, list(range(N_CORES)))
    LAST_RESULTS = res
    out = np.concatenate([res.results[c]["out"] for c in range(N_CORES)], axis=0)
    return np.ascontiguousarray(out.reshape(B, T, IN))


# revision 51
# speedup vs baseline: 1.0127x; 1.0114x over previous
"""Trainium2 Bass kernel for nn_DecoderLSTM.

Key observation: the reference module never reads `features` -- the LSTM input
starts at zeros and is fed back from the predicted point, and h/c start at
zeros.  Every batch row therefore computes the *identical* trajectory
p[t] (t=0..83); the per-row output is just p[t] masked by t < seq_lengths[b].

The sequential 84-step scan is replaced by a parallel-in-time Gauss-Seidel
iteration: all 84 timesteps are updated simultaneously (matmuls with N=84
moving columns), and the linear cell-state recurrence
c_t = sig(f_t)*c_{t-1} + sig(i_t)*tanh(g_t) is solved exactly within each
sweep by the DVE's tensor_tensor_scan.  Convergence (verified against the
host oracle): rel err 2.3e-3 after 4 sweeps, 8.8e-4 after 5, 2.0e-4 after 6.
The sequential version pays the PE weight-load for every 128x128 tile at
every one of 84 steps (~40ns/tile, 216 tiles/step -> ~700us); here each
weight tile is loaded once per sweep and serves all 84 columns.

Hardware rules this code is shaped around:
  - walrus allows ONE sync-wait per instruction.  Tile chains PSUM readers
    with sync edges at tensor granularity, so every PSUM tensor has exactly
    one reader instruction per sweep (a DVE copy/bias-add); ACT never reads
    PSUM and every ACT-written tile is fresh per sweep (ACT-ACT WAW edges
    also emit waits).
  - only one PSUM accumulation group may be open per 2KB bank, so each gate
    region's matmuls are emitted as one tight group.
  - gates are grouped into PSUM tensors by gate type, ordered (g,f01 |
    f23,i | o) so the o-gate matmuls stream while the scan/tanh chain runs:
    the PE's post-cell stall is only gb_o -> sig_o -> h'.

Layouts (per core):
  - states H0s/H1s (ping-pong pair): [128, 4, 85] fp16; col tau holds
    h(tau-1), col 0 is the t=-1 zero state; chunk k on dim1: h[128k+p].
  - x feed Xs: [4, 85] fp16; rows 0:3 = x (col tau = p_{tau-1}), row 3 = 1
    (carries the cell-0 bias through the x matmul, K=4).
  - gate region r = q*4 + ml (gate q of (i,f,o,g), h-chunk ml), 84 columns
    each, distributed across three PSUM tensors per cell in SLOT order
    (12,13,14,15,4,5 | 6,7,0,1,2,3 | 8,9,10,11).
  - weights: lhsT tiles [K=128, M=128] fp16, m-major (free = m*512+k*128+j),
    DMA'd in 4 chunks ordered by first use.
"""

import numpy as np

B = 16384
H = 512
T = 84
IN = 3
N_CORES = 8
NB = B // N_CORES          # 2048 rows per core
M_TILES = 16               # 2048 gate dims / 128
BT = NB // 128             # 16 batch tiles per core
F_OUT = T * IN             # 252
J_SWEEPS = 4

# gate regions in emission (slot) order: tensor A = g gates + f chunks 0,1;
# tensor B = f chunks 2,3 + i gates; tensor O = o gates (streamed last)
SLOTS_A = (12, 13, 14, 15, 4, 5)
SLOTS_B = (6, 7, 0, 1, 2, 3)
SLOTS_O = (8, 9, 10, 11)
SLOTS = SLOTS_A + SLOTS_B + SLOTS_O
# weight DMA chunks (4 m-tiles each) in first-use order of SLOTS
WCHUNKS = ((12, 16), (4, 8), (0, 4), (8, 12))

_COMPILED = None           # nc cache
LAST_RESULTS = None        # BassKernelResults from the last run (for test.py)


def _gate_reorder(a, axis=0):
    """torch gate order (i,f,g,o) -> (i,f,o,g) along `axis` (size 4H)."""
    parts = np.split(a, 4, axis=axis)
    return np.concatenate([parts[0], parts[1], parts[3], parts[2]], axis=axis)


def _lhsT_tiles_mmajor(wT, kt):
    """wT: [K, 2048] -> [128, 16*kt*128] with free index (m, k, j)."""
    K = wT.shape[0]
    assert K == kt * 128
    a = wT.reshape(kt, 128, M_TILES, 128)       # [k, p, m, j]
    return np.ascontiguousarray(a.transpose(1, 2, 0, 3).reshape(128, kt * 2048))


def _build_program():
    import concourse.bass as bass
    import concourse.tile as tile
    import concourse.mybir as mybir
    from contextlib import ExitStack

    f16 = mybir.dt.float16
    f32 = mybir.dt.float32
    AF = mybir.ActivationFunctionType
    Alu = mybir.AluOpType

    class SplitDrainTileContext(tile.TileContext):
        """This container's walrus allows only one sync-wait per instruction;
        Tile's kernel-tail drain carries one wait per live semaphore.  Split
        it into a chain of single-wait drains (same semantics: by the last
        drain every semaphore has reached its target)."""

        def _drain_and_barrier(self, tick_clock, wait_clock):
            from concourse.vector_clock import ScopedClock
            drain_inst = self.nc.sync.drain()
            wait_clock.add_sem_waits(
                drain_inst.ins, ScopedClock({None: tick_clock.global_clock}))
            si = drain_inst.ins.sync_info
            waits = list(si.on_wait or []) if si is not None else []
            if len(waits) > 1:
                ups = list(si.on_update or [])
                drain_inst.ins.sync_info = mybir.SyncInfo(
                    on_wait=[waits[0]], on_update=ups)
                for w in waits[1:]:
                    d2 = self.nc.sync.drain()
                    d2.ins.sync_info = mybir.SyncInfo(on_wait=[w], on_update=[])
            self.nc.all_engine_barrier()
            popped = self.nc._tile_sem_poison_stack.pop()
            assert popped is self._sem_poison
            self.nc.clear_and_free_semaphores(list(self.sems.allocated().values()))
            self.nc.all_engine_barrier()

    nc = bass.Bass()

    w0T = nc.declare_dram_parameter("w0T", [128, 4 * 2048], f16, isOutput=False)
    w1iT = nc.declare_dram_parameter("w1iT", [128, 4 * 2048], f16, isOutput=False)
    w1hT = nc.declare_dram_parameter("w1hT", [128, 4 * 2048], f16, isOutput=False)
    wx4T = nc.declare_dram_parameter("wx4T", [4, 2048], f16, isOutput=False)
    wpT = nc.declare_dram_parameter("wpT", [128, 12], f16, isOutput=False)
    b1rd = nc.declare_dram_parameter("b1rep", [128, M_TILES * T], f16, isOutput=False)
    oh3d = nc.declare_dram_parameter("oh3", [3, 3 * 128], f16, isOutput=False)
    bpcd = nc.declare_dram_parameter("bpc84", [3, T], f32, isOutput=False)
    tvd = nc.declare_dram_parameter("tvals", [1, F_OUT], f16, isOutput=False)
    xsid = nc.declare_dram_parameter("xsinit", [4, T + 1], f16, isOutput=False)
    lensd = nc.declare_dram_parameter("lens", [NB], f16, isOutput=False)
    outd = nc.declare_dram_parameter("out", [NB, F_OUT], f32, isOutput=True)

    with ExitStack() as ctx:
        tc = ctx.enter_context(SplitDrainTileContext(nc))
        const = ctx.enter_context(tc.tile_pool(name="const", bufs=1))
        tmp = ctx.enter_context(tc.tile_pool(name="tmp", bufs=2))
        GA0 = ctx.enter_context(nc.psum_tensor([128, 1, 512], f32))
        GB0 = ctx.enter_context(nc.psum_tensor([128, 1, 512], f32))
        GO0 = ctx.enter_context(nc.psum_tensor([128, 1, 512], f32))
        GA1 = ctx.enter_context(nc.psum_tensor([128, 1, 512], f32))
        GB1 = ctx.enter_context(nc.psum_tensor([128, 1, 512], f32))
        GO1 = ctx.enter_context(nc.psum_tensor([128, 1, 512], f32))
        PB = ctx.enter_context(nc.psum_tensor([128, 1, 512], f32))
        BC = ctx.enter_context(nc.psum_tensor([128, 1, 512], f32))
        G0 = (GA0, GB0, GO0)
        G1 = (GA1, GB1, GO1)
        Pap = PB[0:3, 0, 0:T]              # head output

        _r2slot = {}
        for s, r in enumerate(SLOTS_A):
            _r2slot[r] = (0, s)
        for s, r in enumerate(SLOTS_B):
            _r2slot[r] = (1, s)
        for s, r in enumerate(SLOTS_O):
            _r2slot[r] = (2, s)

        def greg(G, r):
            ti, s = _r2slot[r]
            return G[ti][:, 0, s * T:(s + 1) * T]

        # ---- constants / weights into SBUF ----
        # weight DMAs are chunked and ordered by first use; each chunk's
        # semaphore is absorbed into the PE clock by a tiny ldweights right
        # before its first consuming matmul.
        # DMA ring-flow waits are cumulative per queue and walrus allows
        # only one sync wait per instruction, so the gpsimd queue is
        # reserved for the four output stores; loads alternate between the
        # SP and ACT queues so weight chunks land in parallel and sweeps
        # 0/1 are not DMA-starved (6.4MB of weights vs ~200GB/s per queue).
        queues = (nc.sync.dma_start, nc.scalar.dma_start)
        wx4s = const.tile([4, 2048], f16)
        queues[0](wx4s[:], wx4T[:, :])
        Xs = const.tile([4, T + 1], f16)
        queues[1](Xs[:], xsid[:, :])  # rows 0:3 zero, row 3 = 1 (bias)
        b1rs = const.tile([128, M_TILES * T], f16)
        queues[0](b1rs[:], b1rd[:, :])
        wpss = const.tile([128, 12], f16)
        queues[0](wpss[:], wpT[:, :])
        w1is = const.tile([128, 4 * 2048], f16)
        for qi, (lo, hi) in enumerate(WCHUNKS):
            queues[qi % 2](w1is[:, lo * 512:hi * 512], w1iT[:, lo * 512:hi * 512])
        w0s = const.tile([128, 4 * 2048], f16)
        for qi, (lo, hi) in enumerate(WCHUNKS):
            queues[(qi + 1) % 2](w0s[:, lo * 512:hi * 512], w0T[:, lo * 512:hi * 512])
        w1hs = const.tile([128, 4 * 2048], f16)
        for qi, (lo, hi) in enumerate(WCHUNKS):
            queues[qi % 2](w1hs[:, lo * 512:hi * 512], w1hT[:, lo * 512:hi * 512])
        oh3s = const.tile([3, 3 * 128], f16)
        queues[1](oh3s[:], oh3d[:, :])
        bpcs = const.tile([3, T], f32)
        queues[0](bpcs[:], bpcd[:, :])
        tvs = const.tile([1, F_OUT], f16)
        queues[0](tvs[:], tvd[:, :])
        lenss = const.tile([128, BT], f16)
        queues[1](lenss[:], lensd.rearrange("(m p) -> p m", p=128))

        ones1 = const.tile([1, 128], f16)
        nc.vector.memset(ones1[:], 1.0)

        # states: ping-pong buffers, col 0 = zero state
        H0s = [const.tile([128, 4, T + 1], f16, name=f"h0_{i}") for i in range(2)]
        H1s = [const.tile([128, 4, T + 1], f16, name=f"h1_{i}") for i in range(2)]
        for s in (*H0s, *H1s):
            nc.vector.memset(s[:, :, 0:1], 0.0)

        # DVE absorbers for DVE-consumed const DMAs
        absb = const.tile([1, 5], f32)
        nc.vector.tensor_copy(absb[:, 0:1], b1rs[0:1, 0:1])
        nc.vector.tensor_copy(absb[:, 1:2], bpcs[0:1, 0:1])
        nc.vector.tensor_copy(absb[:, 2:3], tvs[0:1, 0:1])
        nc.vector.tensor_copy(absb[:, 3:4], lenss[0:1, 0:1])
        nc.vector.tensor_copy(absb[:, 4:5], Xs[0:1, 0:1])
        nc.tensor.ldweights(wx4s[0:4, 0:1])

        # broadcast the t-values row across partitions (once)
        tvbc = const.tile([128, F_OUT], f16)
        nc.tensor.matmul(BC[:, 0, F_OUT:2 * F_OUT], lhsT=ones1[:], rhs=tvs[:],
                         start=True, stop=True)
        nc.vector.tensor_copy(tvbc[:], BC[:, 0, F_OUT:2 * F_OUT])
        ones252 = const.tile([128, F_OUT], f16)
        nc.vector.memset(ones252[:], 1.0)
        # per-row length masks (t < len), input-only: computed during the
        # sweeps while the DVE idles under the matmul blocks
        maskt = const.tile([128, BT, F_OUT], f16)

        def emit_masks(lo, hi):
            for n in range(lo, hi):
                nc.vector.scalar_tensor_tensor(
                    maskt[:, n, :], tvbc[:], lenss[:, n:n + 1],
                    ones252[:], Alu.is_lt, Alu.mult)

        def cell_elementwise_a(G, cellno, j, bias):
            """Part 1: after tensors A and B are complete -- tanh(g),
            sig(f,i), u, the c scan and tanh(c).  Runs while the o-gate
            matmuls stream through the PE."""
            GA, GB, _ = G
            gba = tmp.tile([128, 6 * T], f16, tag=f"gba{cellno}", bufs=2)
            if bias is not None:
                nc.vector.tensor_add(gba[:], GA[:, 0, 0:6 * T], bias[:, 0:6 * T])
            else:
                nc.vector.tensor_copy(gba[:], GA[:, 0, 0:6 * T])
            tg = tmp.tile([128, 4 * T], f16, tag=f"tg{cellno}_{j}", bufs=1)
            nc.scalar.activation(tg[:], gba[:, 0:4 * T], AF.Tanh)     # tanh(g)
            sgf01 = tmp.tile([128, 2 * T], f16, tag=f"sgf01_{cellno}_{j}", bufs=1)
            nc.scalar.activation(sgf01[:], gba[:, 4 * T:6 * T], AF.Sigmoid)
            gbb = tmp.tile([128, 6 * T], f16, tag=f"gbb{cellno}", bufs=2)
            if bias is not None:
                nc.vector.tensor_add(gbb[:], GB[:, 0, 0:6 * T], bias[:, 6 * T:12 * T])
            else:
                nc.vector.tensor_copy(gbb[:], GB[:, 0, 0:6 * T])
            # sigmoid of (f23, i01) first: chunk-0's u/scan only needs
            # those, so the c-chain starts before sig(i23) finishes
            sgb = tmp.tile([128, 4 * T], f16, tag=f"sgb{cellno}_{j}", bufs=1)
            nc.scalar.activation(sgb[:], gbb[:, 0:4 * T], AF.Sigmoid)
            sgc = tmp.tile([128, 2 * T], f16, tag=f"sgc{cellno}_{j}", bufs=1)
            nc.scalar.activation(sgc[:], gbb[:, 4 * T:6 * T], AF.Sigmoid)
            # per-chunk pipeline: u -> scan -> tanh(c) for chunk c runs
            # while chunk c+1 is still scanning, so tanh(c) of the last
            # chunk lands just after the last scan instead of after all four
            u = tmp.tile([128, 4 * T], f16, tag=f"u{cellno}", bufs=2)
            cf = tmp.tile([128, 4 * T], f16, tag=f"c{cellno}", bufs=2)
            tcns = []
            for c4, sf in ((0, sgf01[:, 0:T]), (1, sgf01[:, T:2 * T]),
                           (2, sgb[:, 0:T]), (3, sgb[:, T:2 * T])):
                cs = slice(c4 * T, (c4 + 1) * T)
                si_ap = (sgb[:, (2 + c4) * T:(3 + c4) * T] if c4 < 2
                         else sgc[:, (c4 - 2) * T:(c4 - 1) * T])
                nc.vector.tensor_mul(u[:, cs], si_ap, tg[:, cs])  # sig(i)*tanh(g)
                nc.vector.tensor_tensor_scan(
                    cf[:, cs], sf, u[:, cs],
                    0.0, Alu.mult, Alu.add)   # c_t = sig(f_t)*c_{t-1} + u_t
                tcn = tmp.tile([128, T], f16, tag=f"tc{cellno}_{j}_{c4}", bufs=1,
                               name=f"tcn{cellno}_{j}_{c4}")
                nc.scalar.activation(tcn[:], cf[:, cs], AF.Tanh)
                tcns.append(tcn)
            return tcns

        def cell_elementwise_o(G, tcn, h_out, cellno, j, bias):
            """Part 2: after tensor O -- sig(o), h' = sig(o)*tanh(c)."""
            _, _, GO = G
            gbo = tmp.tile([128, 4 * T], f16, tag=f"gbo{cellno}", bufs=2)
            if bias is not None:
                nc.vector.tensor_add(gbo[:], GO[:, 0, 0:4 * T], bias[:, 12 * T:])
            else:
                nc.vector.tensor_copy(gbo[:], GO[:, 0, 0:4 * T])
            sgo = tmp.tile([128, 4 * T], f16, tag=f"sgo{cellno}_{j}", bufs=1)
            nc.scalar.activation(sgo[:], gbo[:], AF.Sigmoid)
            # per-chunk h' so the first W_ih matmul (which consumes chunk 0)
            # can start before chunks 1-3 are written
            for c4 in range(4):
                nc.vector.tensor_mul(h_out[:, c4, 1:T + 1],
                                     sgo[:, c4 * T:(c4 + 1) * T], tcn[c4][:])

        def emit_head(h1buf):
            """P(:, t) = W_pc @ h1(t), then Xs[:, 1:] = P + b_pc (fp16)."""
            for k in range(4):
                nc.tensor.matmul(Pap, lhsT=wpss[:, 3 * k:3 * k + 3],
                                 rhs=h1buf[:, k, 1:T + 1],
                                 start=(k == 0), stop=(k == 3))
            nc.vector.tensor_add(Xs[0:3, 1:T + 1], Pap, bpcs[:])

        def absorb(ws, m):
            # absorb the DMA chunk whose first m-tile is m into the PE clock
            nc.tensor.ldweights(ws[:, m * 512:m * 512 + 1])

        for j in range(J_SWEEPS):
            r, w = j % 2, (j + 1) % 2
            if j > 0:
                if j == 1:
                    absorb(wpss, 0)
                emit_head(H1s[r])           # head of sweep j-1 -> x for sweep j
            # G0 = W_hh0 @ h0(prev, shifted) + W_ih0 @ x + b0 (K=4 x+bias
            # pass); one tight accumulation group per gate region.
            for si, m in enumerate(SLOTS):
                if j == 1 and si in (0, 4, 8, 12):
                    absorb(w0s, (m // 4) * 4)
                for k in range(4):
                    if j > 0:
                        nc.tensor.matmul(
                            greg(G0, m),
                            lhsT=w0s[:, m * 512 + k * 128:m * 512 + (k + 1) * 128],
                            rhs=H0s[r][:, k, 0:T],
                            start=(k == 0), stop=False)
                nc.tensor.matmul(
                    greg(G0, m),
                    lhsT=wx4s[0:4, m * 128:(m + 1) * 128],
                    rhs=Xs[0:4, 0:T],
                    start=(j == 0), stop=True)
                if si == len(SLOTS_A) + len(SLOTS_B) - 1:
                    tcn0 = cell_elementwise_a(G0, 0, j, None)
            cell_elementwise_o(G0, tcn0, H0s[w], 0, j, None)
            # G1 = W_ih1 @ h0(this sweep) + W_hh1 @ h1(prev, shifted)
            for si, m in enumerate(SLOTS):
                if j == 0 and si in (0, 4, 8, 12):
                    absorb(w1is, (m // 4) * 4)
                if j == 1 and si in (0, 4, 8, 12):
                    absorb(w1hs, (m // 4) * 4)
                for k in range(4):
                    nc.tensor.matmul(
                        greg(G1, m),
                        lhsT=w1is[:, m * 512 + k * 128:m * 512 + (k + 1) * 128],
                        rhs=H0s[w][:, k, 1:T + 1],
                        start=(k == 0), stop=(j == 0 and k == 3))
                if j > 0:
                    for k in range(4):
                        nc.tensor.matmul(
                            greg(G1, m),
                            lhsT=w1hs[:, m * 512 + k * 128:m * 512 + (k + 1) * 128],
                            rhs=H1s[r][:, k, 0:T],
                            start=False, stop=(k == 3))
                if si == len(SLOTS_A) + len(SLOTS_B) - 1:
                    tcn1 = cell_elementwise_a(G1, 1, j, b1rs)
            cell_elementwise_o(G1, tcn1, H1s[w], 1, j, b1rs)
            if j < 2:
                emit_masks(j * 8, (j + 1) * 8)

        # final head -> Xs[0:3, 1:85] = final trajectory points (with bias)
        emit_head(H1s[J_SWEEPS % 2])

        # ---- broadcast + mask + store ----
        nc.tensor.ldweights(oh3s[0:3, 0:1])   # absorb oh3 DMA sem into PE
        for i in range(3):
            nc.tensor.matmul(BC[:, 0, i * T:(i + 1) * T],
                             lhsT=oh3s[0:3, i * 128:(i + 1) * 128],
                             rhs=Xs[0:3, 1:T + 1], start=True, stop=True)
        # trajectory replicated x4 (interleaved [t, i] -> col 3t+i), so the
        # masking is 4 full-width fp16 multiplies instead of 16 small ops
        pqr = const.tile([128, 4, T, 3], f16)
        for i in range(3):
            nc.vector.tensor_copy(pqr[:, 0, :, i], BC[:, 0, i * T:(i + 1) * T])
        for c in range(1, 4):
            nc.vector.tensor_copy(pqr[:, c, :, :], pqr[:, 0, :, :])
        ot = const.tile([128, BT * F_OUT], f16)
        out_r = outd.rearrange("(n p) f -> p n f", p=128)
        for n4 in range(4):
            nc.vector.tensor_mul(
                ot[:, n4 * 4 * F_OUT:(n4 + 1) * 4 * F_OUT],
                maskt[:, n4 * 4:(n4 + 1) * 4, :], pqr[:, :, :, :])
            # the gpsimd (software DGE) store casts fp16 -> fp32 in flight;
            # the fp16 path is bit-exact because the trajectory already
            # passed through the fp16 Xs tile
            nc.gpsimd.dma_start(
                out_r[:, n4 * 4:(n4 + 1) * 4, :],
                ot[:, n4 * 4 * F_OUT:(n4 + 1) * 4 * F_OUT])

    return nc


def _prep_inputs(inputs):
    f = lambda k: np.asarray(inputs[k], np.float32)
    Wih0 = _gate_reorder(f("W_ih0"))
    Whh0 = _gate_reorder(f("W_hh0"))
    Wih1 = _gate_reorder(f("W_ih1"))
    Whh1 = _gate_reorder(f("W_hh1"))
    b0 = _gate_reorder(f("b_ih0") + f("b_hh0"))
    b1 = _gate_reorder(f("b_ih1") + f("b_hh1"))
    Wpc = f("W_pc")
    bpc = f("b_pc")

    oh3 = np.zeros((3, 3 * 128), np.float16)
    for i in range(3):
        oh3[i, i * 128:(i + 1) * 128] = 1.0

    # b1 replicated across T in SLOT order: col s*T+t = b1[SLOTS[s]*128+p]
    b1m = b1.reshape(M_TILES, 128)[list(SLOTS)]           # [16 slots, 128]
    b1rep = np.repeat(b1m.T[:, :, None], T, axis=2).reshape(128, M_TILES * T)

    common = {
        "w0T": _lhsT_tiles_mmajor(Whh0.T.copy(), 4).astype(np.float16),
        "w1iT": _lhsT_tiles_mmajor(Wih1.T.copy(), 4).astype(np.float16),
        "w1hT": _lhsT_tiles_mmajor(Whh1.T.copy(), 4).astype(np.float16),
        "wx4T": np.ascontiguousarray(
            np.concatenate([Wih0.T, b0[None, :]], 0)).astype(np.float16),
        "wpT": np.ascontiguousarray(
            Wpc.T.reshape(4, 128, 3).transpose(1, 0, 2).reshape(128, 12)
        ).astype(np.float16),
        "b1rep": np.ascontiguousarray(b1rep).astype(np.float16),
        "oh3": oh3,
        "bpc84": np.ascontiguousarray(np.repeat(bpc[:, None], T, axis=1)),
        "tvals": np.repeat(np.arange(T, dtype=np.float16), IN).reshape(1, F_OUT),
        "xsinit": np.concatenate(
            [np.zeros((3, T + 1), np.float16), np.ones((1, T + 1), np.float16)], 0),
    }
    lens = np.asarray(inputs["seq_lengths"]).astype(np.float16)
    in_maps = []
    for c in range(N_CORES):
        m = dict(common)
        m["lens"] = np.ascontiguousarray(lens[c * NB:(c + 1) * NB])
        in_maps.append(m)
    return in_maps


def kernel(**inputs):
    global _COMPILED, LAST_RESULTS
    from concourse.bass_utils import run_bass_kernel_spmd

    if _COMPILED is None:
        _COMPILED = _build_program()
    nc = _COMPILED

    in_maps = _prep_inputs(inputs)
    res = run_bass_kernel_spmd(nc, in_maps, list(range(N_CORES)))
    LAST_RESULTS = res
    out = np.concatenate([res.results[c]["out"] for c in range(N_CORES)], axis=0)
    return np.ascontiguousarray(out.reshape(B, T, IN))
